# revision 2
# baseline (speedup 1.0000x reference)
"""Multi-head causal attention (batch=1, seq=4096, emb=768, 12 heads) on 8 trn2 cores.

Sharding: sequence-parallel over queries. 32 query blocks of 128 rows; core c owns
global blocks {c, 8+c, 16+c, 24+c} (one per "class" i, global block B = 8i+c).
This makes the SPMD program structurally identical on every core: for the class-i
query block, key blocks 0..8i+7 are processed; causality (which of the last 8 key
blocks are valid / diagonal) is encoded in per-core additive mask DATA.

Per core: PE-transpose x_c -> x_c^T; project Q^T,K^T (transposed layout, heads on
partitions) and V (natural layout); AllGather K^T/V across the 8 cores; then for
each of the 6 head pairs run flash-style attention with unnormalized exp-scores
(scores ~ N(0,1) so no max subtraction needed), accumulating ctx^T and rowsums in
PSUM; normalize via a K=1 broadcast matmul of the reciprocal rowsums; finally the
output projection contracts ctx^T directly as the stationary operand.
"""

import os
import sys

import numpy as np

for _p in ("/opt/trn_rl_repo", "/root/.axon_site/_ro/trn_rl_repo"):
    if os.path.isdir(_p) and _p not in sys.path:
        sys.path.insert(0, _p)

import concourse.bass as bass  # noqa: E402
import concourse.mybir as mybir  # noqa: E402
import concourse.tile as tile  # noqa: E402
from concourse import bacc  # noqa: E402
from concourse.bass_utils import run_bass_kernel_spmd  # noqa: E402

F32 = mybir.dt.float32
SEQ = 4096
EMB = 768
HEADS = 12
HD = 64
NCORES = 8
BLK = 128
NBLK = SEQ // BLK          # 32 global key/query blocks
NCLS = NBLK // NCORES      # 4 classes -> 4 local query blocks per core
QLOC = NCLS * BLK          # 512 query rows per core
PAIRS = HEADS // 2         # 6 head pairs
ECH = EMB // BLK           # 6 contraction chunks of 128
NEG = -1.0e9

_COMPILED = None


def _build_program():
    nc = bacc.Bacc("TRN2", target_bir_lowering=False, debug=False,
                   enable_asserts=True, num_devices=NCORES)

    x_c = nc.dram_tensor("x_c", [QLOC, EMB], F32, kind="ExternalInput").ap()
    wq = nc.dram_tensor("wq", [EMB, EMB], F32, kind="ExternalInput").ap()
    wk = nc.dram_tensor("wk", [EMB, EMB], F32, kind="ExternalInput").ap()
    wv = nc.dram_tensor("wv", [EMB, EMB], F32, kind="ExternalInput").ap()
    wo = nc.dram_tensor("wo", [EMB, EMB], F32, kind="ExternalInput").ap()
    b3 = nc.dram_tensor("b3", [BLK, 3, PAIRS], F32, kind="ExternalInput").ap()
    brow = nc.dram_tensor("brow", [1, 2, EMB], F32, kind="ExternalInput").ap()
    am = nc.dram_tensor("am", [BLK, NBLK, BLK], F32, kind="ExternalInput").ap()
    ident = nc.dram_tensor("ident", [BLK, BLK], F32, kind="ExternalInput").ap()
    out_c = nc.dram_tensor("out", [QLOC, EMB], F32, kind="ExternalOutput").ap()

    KT_ELEMS = EMB * QLOC          # 393216, K^T region [768, 512]
    V_ELEMS = QLOC * EMB           # V region [512, 768]
    SHARD = KT_ELEMS + V_ELEMS

    with tile.TileContext(nc) as tc:
        with tc.tile_pool(name="const", bufs=1) as constp, \
             tc.tile_pool(name="persist", bufs=1) as persist, \
             tc.tile_pool(name="dram", bufs=1, space="DRAM") as dram:

            ident_sb = constp.tile([BLK, BLK], F32)
            nc.sync.dma_start(out=ident_sb[:], in_=ident)
            am_sb = constp.tile([BLK, NBLK, BLK], F32)
            nc.sync.dma_start(out=am_sb[:], in_=am)
            b3_sb = constp.tile([BLK, 3, PAIRS], F32)
            nc.sync.dma_start(out=b3_sb[:], in_=b3)
            brow_sb = constp.tile([1, 2, EMB], F32)
            nc.sync.dma_start(out=brow_sb[:], in_=brow)
            ones_col = constp.tile([BLK, 1], F32)
            nc.vector.memset(ones_col[:], 1.0)
            ones_row = constp.tile([BLK, BLK], F32)
            nc.vector.memset(ones_row[:], 1.0)

            xT = persist.tile([BLK, ECH, QLOC], F32)      # x_c^T, [emb-part, e-chunk, q]
            QT = persist.tile([BLK, PAIRS, QLOC], F32)    # Q^T + bias, heads on partitions
            ctxn = persist.tile([BLK, PAIRS, QLOC], F32)  # normalized ctx^T per pair
            wo_sb = persist.tile([BLK, PAIRS, EMB], F32)
            nc.sync.dma_start(
                out=wo_sb[:], in_=wo.rearrange("(p r) f -> r p f", r=BLK))

            bounce = dram.tile([SHARD], F32)
            gathered = dram.tile([NCORES, SHARD], F32)

            # ---- phase 1: transpose x_c -> xT --------------------------------
            with tc.tile_pool(name="xq", bufs=1) as xqp, \
                 tc.tile_pool(name="tps", bufs=2, space="PSUM") as tpp:
                xq = xqp.tile([BLK, NCLS, EMB], F32)
                nc.sync.dma_start(
                    out=xq[:], in_=x_c.rearrange("(b p) e -> p b e", p=BLK))
                for e in range(ECH):
                    tp_ps = tpp.tile([BLK, QLOC], F32, tag="tp")
                    for rb in range(NCLS):
                        nc.tensor.transpose(
                            tp_ps[:, rb * BLK:(rb + 1) * BLK],
                            xq[:, rb, e * BLK:(e + 1) * BLK], ident_sb[:])
                    nc.vector.tensor_copy(xT[:, e, :], tp_ps[:])

            # ---- phase 2: projections ---------------------------------------
            with tc.tile_pool(name="wslab", bufs=3) as wsl, \
                 tc.tile_pool(name="stage", bufs=3) as stage, \
                 tc.tile_pool(name="wvp", bufs=1) as wvp, \
                 tc.tile_pool(name="qkps", bufs=2, space="PSUM") as qkpsp, \
                 tc.tile_pool(name="vps", bufs=2, space="PSUM") as vpsp:

                for p in range(PAIRS):
                    wqp = wsl.tile([BLK, ECH, BLK], F32, tag="w")
                    nc.sync.dma_start(
                        out=wqp[:],
                        in_=wq[:, p * BLK:(p + 1) * BLK].rearrange(
                            "(e r) f -> r e f", r=BLK))
                    qk_ps = qkpsp.tile([BLK, QLOC], F32, tag="qk")
                    for e in range(ECH):
                        nc.tensor.matmul(qk_ps[:], wqp[:, e, :], xT[:, e, :],
                                         start=(e == 0), stop=(e == ECH - 1))
                    nc.vector.tensor_scalar_add(
                        QT[:, p, :], qk_ps[:], b3_sb[:, 0, p:p + 1])

                    wkp = wsl.tile([BLK, ECH, BLK], F32, tag="w")
                    nc.sync.dma_start(
                        out=wkp[:],
                        in_=wk[:, p * BLK:(p + 1) * BLK].rearrange(
                            "(e r) f -> r e f", r=BLK))
                    kt_ps = qkpsp.tile([BLK, QLOC], F32, tag="qk")
                    for e in range(ECH):
                        nc.tensor.matmul(kt_ps[:], wkp[:, e, :], xT[:, e, :],
                                         start=(e == 0), stop=(e == ECH - 1))
                    kt_sb = stage.tile([BLK, QLOC], F32, tag="kt")
                    nc.vector.tensor_scalar_add(
                        kt_sb[:], kt_ps[:], b3_sb[:, 1, p:p + 1])
                    nc.sync.dma_start(
                        out=bounce[p * BLK * QLOC:(p + 1) * BLK * QLOC].rearrange(
                            "(a f) -> a f", f=QLOC),
                        in_=kt_sb[:])

                wv_sb = wvp.tile([BLK, ECH, EMB], F32)
                nc.sync.dma_start(
                    out=wv_sb[:], in_=wv.rearrange("(e r) f -> r e f", r=BLK))
                for rb in range(NCLS):
                    v_ps = vpsp.tile([BLK, EMB], F32, tag="v")
                    for h0 in (0, 512):
                        h1 = min(h0 + 512, EMB)
                        for e in range(ECH):
                            nc.tensor.matmul(
                                v_ps[:, h0:h1],
                                xT[:, e, rb * BLK:(rb + 1) * BLK],
                                wv_sb[:, e, h0:h1],
                                start=(e == 0), stop=False)
                        nc.tensor.matmul(
                            v_ps[:, h0:h1], ones_row[0:1, 0:BLK],
                            brow_sb[0:1, 0, h0:h1], start=False, stop=True,
                            tile_position=(0, 0))
                    v_sb = stage.tile([BLK, EMB], F32, tag="v")
                    nc.vector.tensor_copy(v_sb[:], v_ps[:])
                    off = KT_ELEMS + rb * BLK * EMB
                    nc.sync.dma_start(
                        out=bounce[off:off + BLK * EMB].rearrange(
                            "(a f) -> a f", f=EMB),
                        in_=v_sb[:])

            # ---- phase 3: AllGather K^T and V -------------------------------
            nc.gpsimd.collective_compute(
                "AllGather", mybir.AluOpType.bypass,
                replica_groups=[list(range(NCORES))],
                ins=[bounce.opt()], outs=[gathered.opt()])

            g_kt = gathered[:, 0:KT_ELEMS].rearrange("r (a f) -> r a f", f=QLOC)
            g_v = gathered[:, KT_ELEMS:SHARD].rearrange("r (a f) -> r a f", f=EMB)

            # ---- phase 4: attention -----------------------------------------
            with tc.tile_pool(name="kv", bufs=3) as kvp, \
                 tc.tile_pool(name="pgp", bufs=3) as pgp, \
                 tc.tile_pool(name="fin", bufs=2) as finp, \
                 tc.tile_pool(name="sgps", bufs=1, space="PSUM") as sgp, \
                 tc.tile_pool(name="ctxps", bufs=1, space="PSUM") as ctxp, \
                 tc.tile_pool(name="rsps", bufs=1, space="PSUM") as rsp:

                GROUPS = [(0, 1, 2), (3, 4, 5), (6, 7)]
                for p in range(PAIRS):
                    ctx_ps = ctxp.tile([BLK, QLOC], F32, tag="ctx")
                    rs_ps = rsp.tile([BLK, QLOC], F32, tag="rs")
                    for w in range(NCLS):
                        o = BLK * w
                        nw = QLOC - o
                        kw = kvp.tile([BLK, NCORES, BLK], F32, tag="kw")
                        nc.sync.dma_start(
                            out=kw[:],
                            in_=g_kt[:, p * BLK:(p + 1) * BLK,
                                     w * BLK:(w + 1) * BLK].rearrange(
                                         "r a f -> a r f"))
                        vw = kvp.tile([BLK, NCORES, BLK], F32, tag="vw")
                        nc.sync.dma_start(
                            out=vw[:],
                            in_=g_v[:, w * BLK:(w + 1) * BLK,
                                    p * BLK:(p + 1) * BLK].rearrange(
                                        "r a f -> a r f"))
                        for grp in GROUPS:
                            n = len(grp)
                            sg = sgp.tile([BLK, 6, 512], F32, tag="sg")
                            for m, s in enumerate(grp):
                                j = NCORES * w + s
                                nc.tensor.matmul(
                                    sg[:, 2 * m, 0:nw], kw[0:64, s, :],
                                    QT[0:64, p, o:QLOC], start=True, stop=True)
                                nc.tensor.matmul(
                                    sg[:, 2 * m + 1, 0:nw], kw[64:BLK, s, :],
                                    QT[64:BLK, p, o:QLOC], start=True, stop=True)
                                nc.vector.tensor_add(
                                    sg[:, 2 * m, 0:BLK], sg[:, 2 * m, 0:BLK],
                                    am_sb[:, j, :])
                                nc.vector.tensor_add(
                                    sg[:, 2 * m + 1, 0:BLK],
                                    sg[:, 2 * m + 1, 0:BLK], am_sb[:, j, :])
                            pg = pgp.tile([BLK, 6, 512], F32, tag="pg")
                            nc.scalar.activation(
                                out=pg[:, 0:2 * n, 0:nw], in_=sg[:, 0:2 * n, 0:nw],
                                func=mybir.ActivationFunctionType.Exp, scale=0.125)
                            for m, s in enumerate(grp):
                                j = NCORES * w + s
                                first = (j == 0)
                                last = (j == NBLK - 1)
                                nc.tensor.matmul(
                                    ctx_ps[0:64, o:QLOC], vw[:, s, 0:64],
                                    pg[:, 2 * m, 0:nw], start=first, stop=last,
                                    tile_position=(0, 0), skip_group_check=True)
                                nc.tensor.matmul(
                                    ctx_ps[64:BLK, o:QLOC], vw[:, s, 64:BLK],
                                    pg[:, 2 * m + 1, 0:nw], start=first, stop=last,
                                    tile_position=(0, 64), skip_group_check=True)
                                nc.tensor.matmul(
                                    rs_ps[0:1, o:QLOC], ones_col[:, :],
                                    pg[:, 2 * m, 0:nw], start=first, stop=last,
                                    tile_position=(0, 0), skip_group_check=True)
                                nc.tensor.matmul(
                                    rs_ps[64:65, o:QLOC], ones_col[:, :],
                                    pg[:, 2 * m + 1, 0:nw], start=first, stop=last,
                                    tile_position=(0, 64), skip_group_check=True)
                    rs_sb = finp.tile([BLK, QLOC], F32, tag="rssb")
                    nc.vector.reciprocal(rs_sb[0:1, :], rs_ps[0:1, :])
                    nc.vector.reciprocal(rs_sb[64:65, :], rs_ps[64:65, :])
                    bc_ps = sgp.tile([BLK, QLOC], F32, tag="sg")
                    nc.tensor.matmul(bc_ps[0:64, :], ones_row[0:1, 0:64],
                                     rs_sb[0:1, :], start=True, stop=True,
                                     tile_position=(0, 0))
                    nc.tensor.matmul(bc_ps[64:BLK, :], ones_row[64:65, 0:64],
                                     rs_sb[64:65, :], start=True, stop=True,
                                     tile_position=(64, 64))
                    bc_sb = finp.tile([BLK, QLOC], F32, tag="bcsb")
                    nc.vector.tensor_copy(bc_sb[:], bc_ps[:])
                    nc.vector.tensor_mul(ctxn[:, p, :], ctx_ps[:], bc_sb[:])

            # ---- phase 5: output projection ---------------------------------
            with tc.tile_pool(name="outp", bufs=2) as outp, \
                 tc.tile_pool(name="ops", bufs=2, space="PSUM") as opsp:
                for rb in range(NCLS):
                    o_ps = opsp.tile([BLK, EMB], F32, tag="o")
                    for h0 in (0, 512):
                        h1 = min(h0 + 512, EMB)
                        for p in range(PAIRS):
                            nc.tensor.matmul(
                                o_ps[:, h0:h1],
                                ctxn[:, p, rb * BLK:(rb + 1) * BLK],
                                wo_sb[:, p, h0:h1],
                                start=(p == 0), stop=False)
                        nc.tensor.matmul(
                            o_ps[:, h0:h1], ones_row[0:1, 0:BLK],
                            brow_sb[0:1, 1, h0:h1], start=False, stop=True,
                            tile_position=(0, 0))
                    o_sb = outp.tile([BLK, EMB], F32, tag="osb")
                    nc.vector.tensor_copy(o_sb[:], o_ps[:])
                    nc.sync.dma_start(
                        out=out_c[rb * BLK:(rb + 1) * BLK, :], in_=o_sb[:])

    nc.compile()
    return nc


def _host_inputs(x, wq, bq, wk, bk, wv, bv, wo, bo):
    x2d = np.ascontiguousarray(np.asarray(x, dtype=np.float32).reshape(SEQ, EMB))
    wq = np.ascontiguousarray(np.asarray(wq, dtype=np.float32))
    wk = np.ascontiguousarray(np.asarray(wk, dtype=np.float32))
    wv = np.ascontiguousarray(np.asarray(wv, dtype=np.float32))
    wo = np.ascontiguousarray(np.asarray(wo, dtype=np.float32))
    bq = np.asarray(bq, dtype=np.float32)
    bk = np.asarray(bk, dtype=np.float32)
    bv = np.asarray(bv, dtype=np.float32)
    bo = np.asarray(bo, dtype=np.float32)

    b3 = np.empty((BLK, 3, PAIRS), dtype=np.float32)
    for p in range(PAIRS):
        b3[:, 0, p] = bq[p * BLK:(p + 1) * BLK]
        b3[:, 1, p] = bk[p * BLK:(p + 1) * BLK]
    b3[:, 2, :] = 0.0
    brow = np.empty((1, 2, EMB), dtype=np.float32)
    brow[0, 0] = bv
    brow[0, 1] = bo
    ident = np.eye(BLK, dtype=np.float32)

    tri = np.where(np.arange(BLK)[:, None] <= np.arange(BLK)[None, :],
                   np.float32(0.0), np.float32(NEG))  # [k, q]: k<=q valid

    in_maps = []
    for c in range(NCORES):
        rows = np.concatenate(
            [np.arange((8 * i + c) * BLK, (8 * i + c + 1) * BLK)
             for i in range(NCLS)])
        x_cc = np.ascontiguousarray(x2d[rows])
        amc = np.zeros((BLK, NBLK, BLK), dtype=np.float32)
        for j in range(NBLK):
            s = j % NCORES
            if s == c:
                amc[:, j, :] = tri
            elif s > c:
                amc[:, j, :] = NEG
        in_maps.append({
            "x_c": x_cc, "wq": wq, "wk": wk, "wv": wv, "wo": wo,
            "b3": b3, "brow": brow, "am": amc, "ident": ident,
        })
    return in_maps


def kernel(x, wq, bq, wk, bk, wv, bv, wo, bo):
    global _COMPILED
    if _COMPILED is None:
        _COMPILED = _build_program()
    nc = _COMPILED
    in_maps = _host_inputs(x, wq, bq, wk, bk, wv, bv, wo, bo)
    res = run_bass_kernel_spmd(nc, in_maps, list(range(NCORES))).results
    out = np.empty((SEQ, EMB), dtype=np.float32)
    for c in range(NCORES):
        oc = res[c]["out"]
        for i in range(NCLS):
            g = 8 * i + c
            out[g * BLK:(g + 1) * BLK] = oc[i * BLK:(i + 1) * BLK]
    return out.reshape(1, SEQ, EMB)
